# revision 2
# baseline (speedup 1.0000x reference)
"""Multi-head self-attention Trainium2 kernel (Bass/Tile), batch-parallel
over 8 NeuronCores. v2: all-bf16 matmuls, single software pipeline.

Problem (hardcoded): B=8, L=1024, D=1024, H=16, hd=64, f32 in/out.
  qkv = x @ w_qkv + b_qkv ; per-head scores = q k^T / 8 ; mask ; softmax ;
  out = (P v) heads-merged @ w_out + b_out.

Sharding: one batch element per core (data parallel); full weights on every
core. No collectives.

Per-core dataflow (all matmuls bf16, f32 PSUM accum):
  - q/k projections dim-major: qkT[m][128 dims, L] = Wqk^T X^T per m-tile.
  - V projection computed DIRECTLY token-major: vT[c][128 keys, 16*65] =
    X W_v per key-chunk c, evacuated into per-head 65-wide slots whose last
    column is pre-set to 1 so the attention matmul also produces the softmax
    denominator. No PE transposes needed.
  - scores^T per head via K=64 matmuls at partition offset 0/64 (no
    zero-padding copies); exp fused on ScalarE (scale=1/8 + mask bias).
  - attnV accumulates po[65, 512] halves over key chunks; Pool engine
    evacuates rows 0:64 -> otr (bf16) and row 64 -> den immediately so the
    PSUM bank recycles fast.
  - normalization deferred: reciprocal_approx_fast on den[2, L], cast to
    bf16, PE broadcast via sel-matmul, one DVE multiply -> ot_fin bf16.
  - out-projection per lq half-tile from ot_fin/wo, bias add on DVE, DMA out.
  - emission order software-pipelines projection work into attention blocks
    so ScalarE exp throughput hides under PE matmul streaming; DMAs are
    emitted in need-order because the descriptor queue is in-order.
"""

import sys

import numpy as np

try:
    import concourse.bass as bass  # noqa: F401
except Exception:  # pragma: no cover - defensive path setup
    for p in ("/opt/trn_rl_repo", "/opt/pypackages"):
        if p not in sys.path:
            sys.path.insert(0, p)
    import concourse.bass as bass  # noqa: F401

from contextlib import ExitStack

import ml_dtypes
import concourse.tile as tile
from concourse import bacc, mybir
from concourse.bass_utils import run_bass_kernel_spmd

F32 = mybir.dt.float32
BF16 = mybir.dt.bfloat16

B, L, D = 8, 1024, 1024
H, HD = 16, 64
D3 = 3 * D
N_CORES = 8
PART = 128
NK = D // PART  # 8 contraction chunks
NLQ = L // PART  # 8 query tiles
NLK = L // PART  # 8 key tiles
SLOT = HD + 1  # 65: head slot width in vT (64 dims + ones column)


def build_nc():
    nc = bacc.Bacc("TRN2", target_bir_lowering=False, debug=False)

    xT = nc.dram_tensor("xT", (NK, PART, L), BF16, kind="ExternalInput").ap()
    # wqk[m, p, k*128+c] = w_qkv[k*128+p, m*128+c], m in 0..15 (q then k)
    wqk = nc.dram_tensor("wqk", (2 * NK, PART, L), BF16, kind="ExternalInput").ap()
    # wv[k, p, vd] = w_qkv[k*128+p, 2D+vd]
    wv = nc.dram_tensor("wv", (NK, PART, D), BF16, kind="ExternalInput").ap()
    # wo[j, p, od] = w_out[j*128+p, od]
    wo_d = nc.dram_tensor("wo", (NK, PART, D), BF16, kind="ExternalInput").ap()
    bqk = nc.dram_tensor("bqk", (PART, 2 * NK), F32, kind="ExternalInput").ap()
    bv = nc.dram_tensor("bv", (PART, D), F32, kind="ExternalInput").ap()
    bout = nc.dram_tensor("bout", (PART, D), F32, kind="ExternalInput").ap()
    maskb = nc.dram_tensor("maskb", (PART, NLK), F32, kind="ExternalInput").ap()
    sel = nc.dram_tensor("sel", (2, PART), BF16, kind="ExternalInput").ap()
    Y = nc.dram_tensor("Y", (L, D), F32, kind="ExternalOutput").ap()

    with tile.TileContext(nc) as tc, ExitStack() as ctx:
        # ---------------- persistent tiles ----------------
        singles = ctx.enter_context(tc.tile_pool(name="singles", bufs=1))
        sel_sb = singles.tile([2, PART], BF16)
        bqk_sb = singles.tile([PART, 2 * NK], F32)
        mb_sb = singles.tile([PART, NLK], F32)
        bv_sb = singles.tile([PART, D], F32)
        bout_sb = singles.tile([PART, D], F32)

        qk_pool = ctx.enter_context(tc.tile_pool(name="qkT", bufs=1))
        qkT = [qk_pool.tile([PART, L], BF16, tag=f"qkT{m}", name=f"qkT{m}") for m in range(2 * NK)]

        vt_pool = ctx.enter_context(tc.tile_pool(name="vT", bufs=1))
        vT = [vt_pool.tile([PART, H * SLOT], BF16, tag=f"vT{c}", name=f"vT{c}") for c in range(NLK)]

        otf_pool = ctx.enter_context(tc.tile_pool(name="otf", bufs=1))
        ot_fin = [otf_pool.tile([PART, L], BF16, tag=f"otf{j}", name=f"otf{j}") for j in range(NK)]

        wo_pool = ctx.enter_context(tc.tile_pool(name="wop", bufs=1))
        wo = [wo_pool.tile([PART, D], BF16, tag=f"wo{k}", name=f"wo{k}") for k in range(NK)]

        xt_pool = ctx.enter_context(tc.tile_pool(name="xt", bufs=1))
        xt = [xt_pool.tile([PART, L], BF16, tag=f"xt{k}", name=f"xt{k}") for k in range(NK)]

        wv_pool = ctx.enter_context(tc.tile_pool(name="wvp", bufs=1))
        wvt = [wv_pool.tile([PART, D], BF16, tag=f"wv{k}", name=f"wv{k}") for k in range(NK)]

        # ---------------- rotating pools ----------------
        wqk_pool = ctx.enter_context(tc.tile_pool(name="wqkp", bufs=4))
        et_pool = ctx.enter_context(tc.tile_pool(name="etp", bufs=18))
        otr_pool = ctx.enter_context(tc.tile_pool(name="otrp", bufs=6))
        dt_pool = ctx.enter_context(tc.tile_pool(name="dtp", bufs=2))
        dnt_pool = ctx.enter_context(tc.tile_pool(name="dntp", bufs=4))
        den_all = singles.tile([H, L], F32, name="den_all")
        rcp_all = singles.tile([H, L], F32, name="rcp_all")
        rcb_all = singles.tile([H, L], BF16, name="rcb_all")
        fs_pool = ctx.enter_context(tc.tile_pool(name="fsp", bufs=2))

        pst_pool = ctx.enter_context(tc.tile_pool(name="pst", bufs=2, space="PSUM"))
        po_pool = ctx.enter_context(tc.tile_pool(name="po", bufs=2, space="PSUM"))
        pj_pool = ctx.enter_context(tc.tile_pool(name="pj", bufs=2, space="PSUM"))

        # ---------------- emission helpers ----------------
        def dma_xt(k):
            nc.sync.dma_start(xt[k][:], xT[k])

        def qk_proj_half(m, nh, wtile):
            """One [128, 512] output half of q/k m-tile m."""
            ns = slice(nh * 512, (nh + 1) * 512)
            pj = pj_pool.tile([PART, 512], F32, tag="pj", name="pj")
            for k in range(NK):
                nc.tensor.matmul(
                    pj[:],
                    wtile[:, k * PART : (k + 1) * PART],
                    xt[k][:, ns],
                    start=(k == 0),
                    stop=(k == NK - 1),
                )
            nc.vector.tensor_scalar_add(qkT[m][:, ns], pj[:], bqk_sb[:, m : m + 1])

        def v_proj_half(c, nh):
            """vT key-chunk c, v-dims half nh -> strided into head slots."""
            ns = slice(nh * 512, (nh + 1) * 512)
            pj = pj_pool.tile([PART, 512], F32, tag="pj", name="pj")
            for k in range(NK):
                nc.tensor.matmul(
                    pj[:],
                    xt[k][:, c * PART : (c + 1) * PART],
                    wvt[k][:, ns],
                    start=(k == 0),
                    stop=(k == NK - 1),
                )
            v3 = vT[c][:].rearrange("p (h w) -> p h w", w=SLOT)
            dst = v3[:, nh * 8 : (nh + 1) * 8, 0:HD]
            src = pj[:].rearrange("p (h w) -> p h w", w=HD)
            bsrc = bv_sb[:, ns].rearrange("p (h w) -> p h w", w=HD)
            nc.vector.tensor_add(dst, src, bsrc)

        def score_chunk(h, c):
            """scores^T chunk c of head h (K=64 at partition offset) + exp."""
            j = h // 2
            ro = (h % 2) * HD
            st = pst_pool.tile([PART, L], F32, tag="pst", name="pst")
            for nh in range(2):
                ns = slice(nh * 512, (nh + 1) * 512)
                nc.tensor.matmul(
                    st[:, ns],
                    qkT[NK + j][ro : ro + HD, c * PART : (c + 1) * PART],
                    qkT[j][ro : ro + HD, ns],
                    start=True,
                    stop=True,
                )
            et = et_pool.tile([PART, L], BF16, tag="et", name="et")
            nc.scalar.activation(
                et[:],
                st[:],
                mybir.ActivationFunctionType.Exp,
                bias=mb_sb[:, c : c + 1],
                scale=1.0 / 8.0,
            )
            return et

        # ================= emission =================
        # --- prologue DMAs, strictly in need-order (queue is in-order) ---
        dma_xt(0)
        w_q0 = wqk_pool.tile([PART, L], BF16, tag="wqk", name="wq0")
        nc.sync.dma_start(w_q0[:], wqk[0])
        w_k0 = wqk_pool.tile([PART, L], BF16, tag="wqk", name="wk0")
        nc.sync.dma_start(w_k0[:], wqk[NK])
        for k in range(1, NK):
            dma_xt(k)
        nc.sync.dma_start(sel_sb[:], sel[:, :])
        nc.sync.dma_start(bqk_sb[:], bqk[:, :])
        nc.sync.dma_start(mb_sb[:], maskb[:, :])
        nc.gpsimd.memset(den_all[:], 1.0)
        # vT ones columns: one strided memset per key-chunk tile
        for c in range(NLK):
            v3 = vT[c][:].rearrange("p (h w) -> p h w", w=SLOT)
            nc.gpsimd.memset(v3[:, :, HD : HD + 1], 1.0)

        # --- fill: qk(0) ---
        for nh in range(2):
            qk_proj_half(0, nh, w_q0)
        w_q1 = wqk_pool.tile([PART, L], BF16, tag="wqk", name="wq1")
        nc.sync.dma_start(w_q1[:], wqk[1])
        w_k1 = wqk_pool.tile([PART, L], BF16, tag="wqk", name="wk1")
        nc.sync.dma_start(w_k1[:], wqk[NK + 1])
        for nh in range(2):
            qk_proj_half(NK + 0, nh, w_k0)
        for k in range(NK):
            nc.sync.dma_start(wvt[k][:], wv[k])
        nc.sync.dma_start(bv_sb[:], bv[:, :])

        # --- fill: scores(0) interleaved with qk(1) ---
        ets = {0: []}
        ets[0].append(score_chunk(0, 0))
        qk_proj_half(1, 0, w_q1)
        ets[0].append(score_chunk(0, 1))
        qk_proj_half(1, 1, w_q1)
        ets[0].append(score_chunk(0, 2))
        qk_proj_half(NK + 1, 0, w_k1)
        ets[0].append(score_chunk(0, 3))
        qk_proj_half(NK + 1, 1, w_k1)
        for c in range(4, NLK):
            ets[0].append(score_chunk(0, c))

        # --- fill: v projection (all 16 halves) ---
        for c in range(NLK):
            for nh in range(2):
                v_proj_half(c, nh)

        # --- steady head blocks ---
        # proj work remaining: qk m-tiles for pairs 2..7, two halves each.
        # DMA for each tile is emitted one unit ahead of its first use.
        proj_ms = []
        for j in range(2, NK):
            proj_ms.append(j)
            proj_ms.append(NK + j)
        proj_units = [(m, nh) for m in proj_ms for nh in range(2)]
        proj_i = 0
        wtiles = {}
        dma_mi = 0

        def prefetch_w():
            nonlocal dma_mi
            if dma_mi < len(proj_ms):
                m = proj_ms[dma_mi]
                wt = wqk_pool.tile([PART, L], BF16, tag="wqk", name="wt")
                nc.sync.dma_start(wt[:], wqk[m])
                wtiles[m] = wt
                dma_mi += 1

        prefetch_w()  # first steady w tile in flight early

        def emit_proj_unit():
            nonlocal proj_i
            if proj_i >= len(proj_units):
                return False
            m, nh = proj_units[proj_i]
            qk_proj_half(m, nh, wtiles[m])
            proj_i += 1
            if nh == 1:
                wtiles.pop(m, None)
                prefetch_w()
            return True

        norm_otr = {}  # j -> otr tile
        norm_rcb2 = {}  # j -> assembled [2, L] recip tile

        def emit_recip_window(lo, hi, cs=slice(0, L)):
            """Batched reciprocal of denominator rows lo:hi (DVE)."""
            with nc.allow_low_precision(reason="softmax recip"):
                nc.vector.reciprocal(rcp_all[lo:hi, cs], den_all[lo:hi, cs])
                nc.vector.tensor_copy(rcb_all[lo:hi, cs], rcp_all[lo:hi, cs])

        def emit_norm_dma(j):
            """Stage 1: assemble pair recip rows at partitions 0:2 (DMA)."""
            rcb2 = dt_pool.tile([2, L], BF16, tag="dt", name="rcb2")
            for s in range(2):
                nc.sync.dma_start(
                    rcb2[s : s + 1, :], rcb_all[2 * j + s : 2 * j + s + 1, :]
                )
            norm_rcb2[j] = rcb2

        def emit_norm_rt(j, pool=None):
            """Stage 2: broadcast via K=2 sel-matmul + DVE multiply."""
            otr_t = norm_otr.pop(j)
            rcb2 = norm_rcb2.pop(j)
            for nh in range(2):
                ns = slice(nh * 512, (nh + 1) * 512)
                rt = (pool or pj_pool).tile([PART, 512], F32, tag="pj" if pool is None else "pst", name="rt")
                nc.tensor.matmul(
                    rt[:], sel_sb[:], rcb2[0:2, ns], start=True, stop=True
                )
                nc.vector.tensor_mul(ot_fin[j][:, ns], otr_t[:, ns], rt[:])

        otr_cur = None
        wo_dma_done = False
        norm_dma_i = [0]
        norm_rt_i = [0]
        for h in range(H):
            j = h // 2
            ro = (h % 2) * HD
            if h % 2 == 0:
                otr_cur = otr_pool.tile([PART, L], BF16, tag="otr", name="otr")
            if h + 1 < H:
                ets[h + 1] = []
            po_tiles = [
                po_pool.tile([SLOT, 512], F32, tag="po", name="po0"),
                po_pool.tile([SLOT, 512], F32, tag="po", name="po1"),
            ]
            for c in range(NLK):
                if h + 1 < H:
                    ets[h + 1].append(score_chunk(h + 1, c))
                for nh in range(2):
                    nc.tensor.matmul(
                        po_tiles[nh][:],
                        vT[c][:, h * SLOT : (h + 1) * SLOT],
                        ets[h][c][:, nh * 512 : (nh + 1) * 512],
                        start=(c == 0),
                        stop=(c == NLK - 1),
                    )
                if c == 2 or c == 5:
                    if not emit_proj_unit() and not wo_dma_done:
                        # proj stream exhausted: queue tail-phase DMAs now
                        for k in range(NK):
                            nc.sync.dma_start(wo[k][:], wo_d[k])
                        nc.sync.dma_start(bout_sb[:], bout[:, :])
                        wo_dma_done = True
                if h == 12 and (c == 1 or c == 5):
                    half = 0 if c == 1 else 1
                    emit_recip_window(0, 12, slice(half * 512, (half + 1) * 512))
                if h in (14, 15) and c == 1:
                    half = 0 if h == 14 else 1
                    emit_recip_window(0, H, slice(half * 512, (half + 1) * 512))
                if h >= 12 and (c == 3 or c == 6):
                    slot = (h - 12) * 2 + (0 if c == 3 else 1)
                    if norm_dma_i[0] <= 5 and norm_dma_i[0] == slot - 1:
                        emit_norm_dma(norm_dma_i[0])
                        norm_dma_i[0] += 1
                    if norm_rt_i[0] <= 4 and norm_rt_i[0] == slot - 3:
                        emit_norm_rt(norm_rt_i[0])
                        norm_rt_i[0] += 1
            ets.pop(h)
            # evacuate po fast: DVE moves rows 0:64 -> otr; the denominator
            # row goes to den_all via DMA (PSUM -> SBUF, any partition).
            for nh in range(2):
                ns = slice(nh * 512, (nh + 1) * 512)
                nc.vector.tensor_copy(otr_cur[ro : ro + HD, ns], po_tiles[nh][0:HD, :])
                dnt = dnt_pool.tile([1, 512], F32, tag="dnt", name="dnt")
                nc.vector.tensor_copy(dnt[:], po_tiles[nh][HD : HD + 1, :])
                nc.sync.dma_start(den_all[h : h + 1, ns], dnt[:])
            if h % 2 == 1:
                norm_otr[j] = otr_cur
            if h == H - 1:
                # rows 14,15 become valid only here; earlier windows read the
                # pre-memset placeholder for them and are redone now
                emit_recip_window(0, H)

        # ================= tail: norms 5,6,7 overlapped with out-proj ===
        emit_norm_rt(5)
        emit_norm_dma(6)
        emit_norm_rt(6, pool=pst_pool)

        def outproj_partial(lq, nh, pool, khi):
            ns = slice(nh * 512, (nh + 1) * 512)
            pf = pool.tile([PART, 512], F32, tag="pj", name="pf")
            for k in range(khi):
                nc.tensor.matmul(
                    pf[:],
                    ot_fin[k][:, lq * PART : (lq + 1) * PART],
                    wo[k][:, ns],
                    start=(k == 0),
                    stop=False,
                )
            return pf

        def outproj_finish(lq, nh, pf, fs):
            ns = slice(nh * 512, (nh + 1) * 512)
            for k in (6, 7):
                nc.tensor.matmul(
                    pf[:],
                    ot_fin[k][:, lq * PART : (lq + 1) * PART],
                    wo[k][:, ns],
                    start=False,
                    stop=(k == 7),
                )
            nc.vector.tensor_add(fs[:, ns], pf[:], bout_sb[:, ns])

        # partial accumulations k0..5 while the final recip window drains
        pfs = {}
        for lq, nh in [(0, 0), (0, 1)]:
            pfs[(lq, nh)] = outproj_partial(lq, nh, pj_pool, 6)
        emit_norm_dma(7)
        emit_norm_rt(7, pool=pst_pool)
        for lq in (0,):
            fs = fs_pool.tile([PART, D], F32, tag="fs", name="fs")
            for nh in range(2):
                outproj_finish(lq, nh, pfs[(lq, nh)], fs)
            nc.sync.dma_start(Y[lq * PART : (lq + 1) * PART, :], fs[:])
        for lq in range(1, NLQ):
            fs = fs_pool.tile([PART, D], F32, tag="fs", name="fs")
            for nh in range(2):
                ns = slice(nh * 512, (nh + 1) * 512)
                pf = pj_pool.tile([PART, 512], F32, tag="pj", name="pf")
                for k in range(NK):
                    nc.tensor.matmul(
                        pf[:],
                        ot_fin[k][:, lq * PART : (lq + 1) * PART],
                        wo[k][:, ns],
                        start=(k == 0),
                        stop=(k == NK - 1),
                    )
                nc.vector.tensor_add(fs[:, ns], pf[:], bout_sb[:, ns])
            nc.sync.dma_start(Y[lq * PART : (lq + 1) * PART, :], fs[:])

    nc.compile()
    return nc


_NC_CACHE = None


def _get_nc():
    global _NC_CACHE
    if _NC_CACHE is None:
        _NC_CACHE = build_nc()
    return _NC_CACHE


def make_in_maps(x, attn_mask, w_qkv, b_qkv, w_out, b_out):
    """Host-side sharding + layout prep -> per-core input maps."""
    bf16 = ml_dtypes.bfloat16
    x = np.asarray(x, dtype=np.float32)
    attn_mask = np.asarray(attn_mask)
    w_qkv = np.asarray(w_qkv, dtype=np.float32)
    b_qkv = np.asarray(b_qkv, dtype=np.float32)
    w_out = np.asarray(w_out, dtype=np.float32)
    b_out = np.asarray(b_out, dtype=np.float32)

    # wqk[m, p, k*128+c] = w_qkv[k*128+p, m*128+c] for m-tiles 0..15 (q, k)
    wqk = np.ascontiguousarray(
        w_qkv[:, : 2 * D]
        .reshape(NK, PART, 2 * NK, PART)
        .transpose(2, 1, 0, 3)
        .reshape(2 * NK, PART, L)
    ).astype(bf16)
    wv = np.ascontiguousarray(w_qkv[:, 2 * D :].reshape(NK, PART, D)).astype(bf16)
    wo = np.ascontiguousarray(w_out.reshape(NK, PART, D)).astype(bf16)
    bqk_h = np.ascontiguousarray(b_qkv[: 2 * D].reshape(2 * NK, PART).T).astype(
        np.float32
    )
    bv_h = np.ascontiguousarray(np.broadcast_to(b_qkv[2 * D :], (PART, D))).astype(
        np.float32
    )
    boutb = np.ascontiguousarray(np.broadcast_to(b_out, (PART, D))).astype(np.float32)
    maskbias = np.where(attn_mask.astype(bool), 0.0, -10000.0).astype(np.float32)

    sel_host = np.zeros((2, PART), dtype=np.float32)
    sel_host[0, 0:HD] = 1.0
    sel_host[1, HD:PART] = 1.0
    sel_host = sel_host.astype(bf16)

    in_maps = []
    for b in range(B):
        xTb = np.ascontiguousarray(x[b].T.reshape(NK, PART, L)).astype(bf16)
        mb = np.ascontiguousarray(maskbias[b].reshape(NLK, PART).T).astype(np.float32)
        in_maps.append(
            {
                "xT": xTb,
                "wqk": wqk,
                "wv": wv,
                "wo": wo,
                "bqk": bqk_h,
                "bv": bv_h,
                "bout": boutb,
                "maskb": mb,
                "sel": sel_host,
            }
        )
    return in_maps


def kernel(x, attn_mask, w_qkv, b_qkv, w_out, b_out):
    in_maps = make_in_maps(x, attn_mask, w_qkv, b_qkv, w_out, b_out)
    nc = _get_nc()
    res = run_bass_kernel_spmd(nc, in_maps, core_ids=list(range(N_CORES)))
    return np.stack([res.results[b]["Y"] for b in range(B)], axis=0)


if __name__ == "__main__":
    rng = np.random.default_rng(0)
    inputs = {
        "x": rng.standard_normal((B, L, D), dtype=np.float32),
        "attn_mask": np.ones((B, L), dtype=bool),
        "w_qkv": ((rng.random((D, D3), dtype=np.float32) - 0.5) / 16.0),
        "b_qkv": np.zeros((D3,), dtype=np.float32),
        "w_out": ((rng.random((D, D), dtype=np.float32) - 0.5) / 16.0),
        "b_out": np.zeros((D,), dtype=np.float32),
    }
    y = kernel(**inputs)
    print(y.shape, y.dtype)


# revision 3
# speedup vs baseline: 1.0017x; 1.0017x over previous
"""Multi-head self-attention Trainium2 kernel (Bass/Tile), batch-parallel
over 8 NeuronCores. v2: all-bf16 matmuls, single software pipeline.

Problem (hardcoded): B=8, L=1024, D=1024, H=16, hd=64, f32 in/out.
  qkv = x @ w_qkv + b_qkv ; per-head scores = q k^T / 8 ; mask ; softmax ;
  out = (P v) heads-merged @ w_out + b_out.

Sharding: one batch element per core (data parallel); full weights on every
core. No collectives.

Per-core dataflow (all matmuls bf16, f32 PSUM accum):
  - q/k projections dim-major: qkT[m][128 dims, L] = Wqk^T X^T per m-tile.
  - V projection computed DIRECTLY token-major: vT[c][128 keys, 16*65] =
    X W_v per key-chunk c, evacuated into per-head 65-wide slots whose last
    column is pre-set to 1 so the attention matmul also produces the softmax
    denominator. No PE transposes needed.
  - scores^T per head via K=64 matmuls at partition offset 0/64 (no
    zero-padding copies); exp fused on ScalarE (scale=1/8 + mask bias).
  - attnV accumulates po[65, 512] halves over key chunks; Pool engine
    evacuates rows 0:64 -> otr (bf16) and row 64 -> den immediately so the
    PSUM bank recycles fast.
  - normalization deferred: reciprocal_approx_fast on den[2, L], cast to
    bf16, PE broadcast via sel-matmul, one DVE multiply -> ot_fin bf16.
  - out-projection per lq half-tile from ot_fin/wo, bias add on DVE, DMA out.
  - emission order software-pipelines projection work into attention blocks
    so ScalarE exp throughput hides under PE matmul streaming; DMAs are
    emitted in need-order because the descriptor queue is in-order.
"""

import sys

import numpy as np

try:
    import concourse.bass as bass  # noqa: F401
except Exception:  # pragma: no cover - defensive path setup
    for p in ("/opt/trn_rl_repo", "/opt/pypackages"):
        if p not in sys.path:
            sys.path.insert(0, p)
    import concourse.bass as bass  # noqa: F401

from contextlib import ExitStack

import ml_dtypes
import concourse.tile as tile
from concourse import bacc, mybir
from concourse.bass_utils import run_bass_kernel_spmd

F32 = mybir.dt.float32
BF16 = mybir.dt.bfloat16

B, L, D = 8, 1024, 1024
H, HD = 16, 64
D3 = 3 * D
N_CORES = 8
PART = 128
NK = D // PART  # 8 contraction chunks
NLQ = L // PART  # 8 query tiles
NLK = L // PART  # 8 key tiles
SLOT = HD + 1  # 65: head slot width in vT (64 dims + ones column)


def build_nc():
    nc = bacc.Bacc("TRN2", target_bir_lowering=False, debug=False)

    xT = nc.dram_tensor("xT", (NK, PART, L), BF16, kind="ExternalInput").ap()
    # wqk[m, p, k*128+c] = w_qkv[k*128+p, m*128+c], m in 0..15 (q then k)
    wqk = nc.dram_tensor("wqk", (2 * NK, PART, L), BF16, kind="ExternalInput").ap()
    # wv[k, p, vd] = w_qkv[k*128+p, 2D+vd]
    wv = nc.dram_tensor("wv", (NK, PART, D), BF16, kind="ExternalInput").ap()
    # wo[j, p, od] = w_out[j*128+p, od]
    wo_d = nc.dram_tensor("wo", (NK, PART, D), BF16, kind="ExternalInput").ap()
    bqk = nc.dram_tensor("bqk", (PART, 2 * NK), F32, kind="ExternalInput").ap()
    bv = nc.dram_tensor("bv", (PART, D), F32, kind="ExternalInput").ap()
    bout = nc.dram_tensor("bout", (PART, D), F32, kind="ExternalInput").ap()
    maskb = nc.dram_tensor("maskb", (PART, NLK), F32, kind="ExternalInput").ap()
    sel = nc.dram_tensor("sel", (2, PART), BF16, kind="ExternalInput").ap()
    Y = nc.dram_tensor("Y", (L, D), F32, kind="ExternalOutput").ap()

    with tile.TileContext(nc) as tc, ExitStack() as ctx:
        # ---------------- persistent tiles ----------------
        singles = ctx.enter_context(tc.tile_pool(name="singles", bufs=1))
        sel_sb = singles.tile([2, PART], BF16)
        bqk_sb = singles.tile([PART, 2 * NK], F32)
        mb_sb = singles.tile([PART, NLK], F32)
        bv_sb = singles.tile([PART, D], F32)
        bout_sb = singles.tile([PART, D], F32)

        qk_pool = ctx.enter_context(tc.tile_pool(name="qkT", bufs=1))
        qkT = [qk_pool.tile([PART, L], BF16, tag=f"qkT{m}", name=f"qkT{m}") for m in range(2 * NK)]

        vt_pool = ctx.enter_context(tc.tile_pool(name="vT", bufs=1))
        vT = [vt_pool.tile([PART, H * SLOT], BF16, tag=f"vT{c}", name=f"vT{c}") for c in range(NLK)]

        otf_pool = ctx.enter_context(tc.tile_pool(name="otf", bufs=1))
        ot_fin = [otf_pool.tile([PART, L], BF16, tag=f"otf{j}", name=f"otf{j}") for j in range(NK)]

        wo_pool = ctx.enter_context(tc.tile_pool(name="wop", bufs=1))
        wo = [wo_pool.tile([PART, D], BF16, tag=f"wo{k}", name=f"wo{k}") for k in range(NK)]

        xt_pool = ctx.enter_context(tc.tile_pool(name="xt", bufs=1))
        xt = [xt_pool.tile([PART, L], BF16, tag=f"xt{k}", name=f"xt{k}") for k in range(NK)]

        wv_pool = ctx.enter_context(tc.tile_pool(name="wvp", bufs=1))
        wvt = [wv_pool.tile([PART, D], BF16, tag=f"wv{k}", name=f"wv{k}") for k in range(NK)]

        # ---------------- rotating pools ----------------
        wqk_pool = ctx.enter_context(tc.tile_pool(name="wqkp", bufs=4))
        et_pool = ctx.enter_context(tc.tile_pool(name="etp", bufs=18))
        otr_pool = ctx.enter_context(tc.tile_pool(name="otrp", bufs=6))
        dt_pool = ctx.enter_context(tc.tile_pool(name="dtp", bufs=2))
        dnt_pool = ctx.enter_context(tc.tile_pool(name="dntp", bufs=4))
        den_all = singles.tile([H, L], F32, name="den_all")
        rcp_all = singles.tile([H, L], F32, name="rcp_all")
        rcb_all = singles.tile([H, L], BF16, name="rcb_all")
        fs_pool = ctx.enter_context(tc.tile_pool(name="fsp", bufs=2))

        pst_pool = ctx.enter_context(tc.tile_pool(name="pst", bufs=2, space="PSUM"))
        po_pool = ctx.enter_context(tc.tile_pool(name="po", bufs=2, space="PSUM"))
        pj_pool = ctx.enter_context(tc.tile_pool(name="pj", bufs=2, space="PSUM"))

        # ---------------- emission helpers ----------------
        def dma_xt(k):
            nc.sync.dma_start(xt[k][:], xT[k])

        def qk_proj_half(m, nh, wtile):
            """One [128, 512] output half of q/k m-tile m."""
            ns = slice(nh * 512, (nh + 1) * 512)
            pj = pj_pool.tile([PART, 512], F32, tag="pj", name="pj")
            for k in range(NK):
                nc.tensor.matmul(
                    pj[:],
                    wtile[:, k * PART : (k + 1) * PART],
                    xt[k][:, ns],
                    start=(k == 0),
                    stop=(k == NK - 1),
                )
            nc.vector.tensor_scalar_add(qkT[m][:, ns], pj[:], bqk_sb[:, m : m + 1])

        def v_proj_half(c, nh):
            """vT key-chunk c, v-dims half nh -> strided into head slots."""
            ns = slice(nh * 512, (nh + 1) * 512)
            pj = pj_pool.tile([PART, 512], F32, tag="pj", name="pj")
            for k in range(NK):
                nc.tensor.matmul(
                    pj[:],
                    xt[k][:, c * PART : (c + 1) * PART],
                    wvt[k][:, ns],
                    start=(k == 0),
                    stop=(k == NK - 1),
                )
            v3 = vT[c][:].rearrange("p (h w) -> p h w", w=SLOT)
            dst = v3[:, nh * 8 : (nh + 1) * 8, 0:HD]
            src = pj[:].rearrange("p (h w) -> p h w", w=HD)
            bsrc = bv_sb[:, ns].rearrange("p (h w) -> p h w", w=HD)
            nc.vector.tensor_add(dst, src, bsrc)

        def score_chunk(h, c):
            """scores^T chunk c of head h (K=64 at partition offset) + exp."""
            j = h // 2
            ro = (h % 2) * HD
            st = pst_pool.tile([PART, L], F32, tag="pst", name="pst")
            for nh in range(2):
                ns = slice(nh * 512, (nh + 1) * 512)
                nc.tensor.matmul(
                    st[:, ns],
                    qkT[NK + j][ro : ro + HD, c * PART : (c + 1) * PART],
                    qkT[j][ro : ro + HD, ns],
                    start=True,
                    stop=True,
                )
            et = et_pool.tile([PART, L], BF16, tag="et", name="et")
            nc.scalar.activation(
                et[:],
                st[:],
                mybir.ActivationFunctionType.Exp,
                bias=mb_sb[:, c : c + 1],
                scale=1.0 / 8.0,
            )
            return et

        # ================= emission =================
        # --- prologue DMAs, strictly in need-order (queue is in-order) ---
        dma_xt(0)
        w_q0 = wqk_pool.tile([PART, L], BF16, tag="wqk", name="wq0")
        nc.sync.dma_start(w_q0[:], wqk[0])
        w_k0 = wqk_pool.tile([PART, L], BF16, tag="wqk", name="wk0")
        nc.sync.dma_start(w_k0[:], wqk[NK])
        for k in range(1, NK):
            dma_xt(k)
        nc.sync.dma_start(sel_sb[:], sel[:, :])
        nc.sync.dma_start(bqk_sb[:], bqk[:, :])
        nc.sync.dma_start(mb_sb[:], maskb[:, :])
        nc.gpsimd.memset(den_all[:], 1.0)
        # vT ones columns: one strided memset per key-chunk tile
        for c in range(NLK):
            v3 = vT[c][:].rearrange("p (h w) -> p h w", w=SLOT)
            nc.gpsimd.memset(v3[:, :, HD : HD + 1], 1.0)

        # --- fill: qk(0) ---
        for nh in range(2):
            qk_proj_half(0, nh, w_q0)
        w_q1 = wqk_pool.tile([PART, L], BF16, tag="wqk", name="wq1")
        nc.sync.dma_start(w_q1[:], wqk[1])
        w_k1 = wqk_pool.tile([PART, L], BF16, tag="wqk", name="wk1")
        nc.sync.dma_start(w_k1[:], wqk[NK + 1])
        for nh in range(2):
            qk_proj_half(NK + 0, nh, w_k0)
        for k in range(NK):
            nc.sync.dma_start(wvt[k][:], wv[k])
        nc.sync.dma_start(bv_sb[:], bv[:, :])
        for k in range(NK):
            nc.sync.dma_start(wo[k][:], wo_d[k])
        nc.sync.dma_start(bout_sb[:], bout[:, :])

        # --- fill: scores(0) interleaved with qk(1) ---
        ets = {0: []}
        ets[0].append(score_chunk(0, 0))
        qk_proj_half(1, 0, w_q1)
        ets[0].append(score_chunk(0, 1))
        qk_proj_half(1, 1, w_q1)
        ets[0].append(score_chunk(0, 2))
        qk_proj_half(NK + 1, 0, w_k1)
        ets[0].append(score_chunk(0, 3))
        qk_proj_half(NK + 1, 1, w_k1)
        for c in range(4, NLK):
            ets[0].append(score_chunk(0, c))

        # --- fill: v projection (all 16 halves) ---
        for c in range(NLK):
            for nh in range(2):
                v_proj_half(c, nh)

        # --- steady head blocks ---
        # proj work remaining: qk m-tiles for pairs 2..7, two halves each.
        # DMA for each tile is emitted one unit ahead of its first use.
        proj_ms = []
        for j in range(2, NK):
            proj_ms.append(j)
            proj_ms.append(NK + j)
        proj_units = [(m, nh) for m in proj_ms for nh in range(2)]
        proj_i = 0
        wtiles = {}
        dma_mi = 0

        def prefetch_w():
            nonlocal dma_mi
            if dma_mi < len(proj_ms):
                m = proj_ms[dma_mi]
                wt = wqk_pool.tile([PART, L], BF16, tag="wqk", name="wt")
                nc.sync.dma_start(wt[:], wqk[m])
                wtiles[m] = wt
                dma_mi += 1

        prefetch_w()  # first steady w tile in flight early

        def emit_proj_unit():
            nonlocal proj_i
            if proj_i >= len(proj_units):
                return False
            m, nh = proj_units[proj_i]
            qk_proj_half(m, nh, wtiles[m])
            proj_i += 1
            if nh == 1:
                wtiles.pop(m, None)
                prefetch_w()
            return True

        norm_otr = {}  # j -> otr tile
        norm_rcb2 = {}  # j -> assembled [2, L] recip tile

        def emit_recip_window(lo, hi, cs=slice(0, L)):
            """Batched reciprocal of denominator rows lo:hi (DVE)."""
            with nc.allow_low_precision(reason="softmax recip"):
                nc.vector.reciprocal(rcp_all[lo:hi, cs], den_all[lo:hi, cs])
                nc.vector.tensor_copy(rcb_all[lo:hi, cs], rcp_all[lo:hi, cs])

        def emit_norm_dma(j):
            """Stage 1: assemble pair recip rows at partitions 0:2 (DMA)."""
            rcb2 = dt_pool.tile([2, L], BF16, tag="dt", name="rcb2")
            for s in range(2):
                nc.sync.dma_start(
                    rcb2[s : s + 1, :], rcb_all[2 * j + s : 2 * j + s + 1, :]
                )
            norm_rcb2[j] = rcb2

        def emit_norm_rt(j, pool=None):
            """Stage 2: broadcast via K=2 sel-matmul + DVE multiply."""
            otr_t = norm_otr.pop(j)
            rcb2 = norm_rcb2.pop(j)
            for nh in range(2):
                ns = slice(nh * 512, (nh + 1) * 512)
                rt = (pool or pj_pool).tile([PART, 512], F32, tag="pj" if pool is None else "pst", name="rt")
                nc.tensor.matmul(
                    rt[:], sel_sb[:], rcb2[0:2, ns], start=True, stop=True
                )
                nc.vector.tensor_mul(ot_fin[j][:, ns], otr_t[:, ns], rt[:])

        otr_cur = None
        wo_dma_done = False
        norm_dma_i = [0]
        norm_rt_i = [0]
        for h in range(H):
            j = h // 2
            ro = (h % 2) * HD
            if h % 2 == 0:
                otr_cur = otr_pool.tile([PART, L], BF16, tag="otr", name="otr")
            if h + 1 < H:
                ets[h + 1] = []
            po_tiles = [
                po_pool.tile([SLOT, 512], F32, tag="po", name="po0"),
                po_pool.tile([SLOT, 512], F32, tag="po", name="po1"),
            ]
            for c in range(NLK):
                if h + 1 < H:
                    ets[h + 1].append(score_chunk(h + 1, c))
                for nh in range(2):
                    nc.tensor.matmul(
                        po_tiles[nh][:],
                        vT[c][:, h * SLOT : (h + 1) * SLOT],
                        ets[h][c][:, nh * 512 : (nh + 1) * 512],
                        start=(c == 0),
                        stop=(c == NLK - 1),
                    )
                if c == 2 or c == 5:
                    emit_proj_unit()
                if h == 12 and (c == 1 or c == 5):
                    half = 0 if c == 1 else 1
                    emit_recip_window(0, 12, slice(half * 512, (half + 1) * 512))
                if h in (14, 15) and c == 1:
                    half = 0 if h == 14 else 1
                    emit_recip_window(0, H, slice(half * 512, (half + 1) * 512))
                if h >= 12 and (c == 3 or c == 6):
                    slot = (h - 12) * 2 + (0 if c == 3 else 1)
                    if norm_dma_i[0] <= 5 and norm_dma_i[0] == slot - 1:
                        emit_norm_dma(norm_dma_i[0])
                        norm_dma_i[0] += 1
                    if norm_rt_i[0] <= 4 and norm_rt_i[0] == slot - 3:
                        emit_norm_rt(norm_rt_i[0])
                        norm_rt_i[0] += 1
            ets.pop(h)
            # evacuate po fast: DVE moves rows 0:64 -> otr; the denominator
            # row goes to den_all via DMA (PSUM -> SBUF, any partition).
            for nh in range(2):
                ns = slice(nh * 512, (nh + 1) * 512)
                nc.vector.tensor_copy(otr_cur[ro : ro + HD, ns], po_tiles[nh][0:HD, :])
                dnt = dnt_pool.tile([1, 512], F32, tag="dnt", name="dnt")
                nc.vector.tensor_copy(dnt[:], po_tiles[nh][HD : HD + 1, :])
                nc.sync.dma_start(den_all[h : h + 1, ns], dnt[:])
            if h % 2 == 1:
                norm_otr[j] = otr_cur


        # ================= tail: norms 5,6,7 overlapped with out-proj ===
        emit_norm_rt(5)
        emit_norm_dma(6)
        emit_norm_rt(6, pool=pst_pool)
        rcb27 = dt_pool.tile([2, L], BF16, tag="dt", name="rcb27")

        def outproj_partial(lq, nh, pool, khi):
            ns = slice(nh * 512, (nh + 1) * 512)
            pf = pool.tile([PART, 512], F32, tag="pj", name="pf")
            for k in range(khi):
                nc.tensor.matmul(
                    pf[:],
                    ot_fin[k][:, lq * PART : (lq + 1) * PART],
                    wo[k][:, ns],
                    start=(k == 0),
                    stop=False,
                )
            return pf

        def outproj_finish(lq, nh, pf, fs):
            ns = slice(nh * 512, (nh + 1) * 512)
            for k in (6, 7):
                nc.tensor.matmul(
                    pf[:],
                    ot_fin[k][:, lq * PART : (lq + 1) * PART],
                    wo[k][:, ns],
                    start=False,
                    stop=(k == 7),
                )
            nc.vector.tensor_add(fs[:, ns], pf[:], bout_sb[:, ns])

        # partial accumulations k0..5 while the final recip window drains,
        # pipelined against the column-split final window
        pfs = {}
        otr7 = norm_otr.pop(7)
        emit_recip_window(0, H, slice(0, 512))
        for s in range(2):
            nc.sync.dma_start(
                rcb27[s : s + 1, 0:512], rcb_all[14 + s : 15 + s, 0:512]
            )
        for lq, nh in [(0, 0), (0, 1)]:
            pfs[(lq, nh)] = outproj_partial(lq, nh, pj_pool, 6)
        emit_recip_window(0, H, slice(512, 1024))
        for s in range(2):
            nc.sync.dma_start(
                rcb27[s : s + 1, 512:1024], rcb_all[14 + s : 15 + s, 512:1024]
            )
        for nh in range(2):
            ns = slice(nh * 512, (nh + 1) * 512)
            rt = pst_pool.tile([PART, 512], F32, tag="pst", name="rt7")
            nc.tensor.matmul(
                rt[:], sel_sb[:], rcb27[0:2, ns], start=True, stop=True
            )
            nc.vector.tensor_mul(ot_fin[7][:, ns], otr7[:, ns], rt[:])
        for lq in (0,):
            fs = fs_pool.tile([PART, D], F32, tag="fs", name="fs")
            for nh in range(2):
                outproj_finish(lq, nh, pfs[(lq, nh)], fs)
            nc.sync.dma_start(Y[lq * PART : (lq + 1) * PART, :], fs[:])
        for lq in range(1, NLQ):
            fs = fs_pool.tile([PART, D], F32, tag="fs", name="fs")
            for nh in range(2):
                ns = slice(nh * 512, (nh + 1) * 512)
                pf = pj_pool.tile([PART, 512], F32, tag="pj", name="pf")
                for k in range(NK):
                    nc.tensor.matmul(
                        pf[:],
                        ot_fin[k][:, lq * PART : (lq + 1) * PART],
                        wo[k][:, ns],
                        start=(k == 0),
                        stop=(k == NK - 1),
                    )
                nc.vector.tensor_add(fs[:, ns], pf[:], bout_sb[:, ns])
            nc.sync.dma_start(Y[lq * PART : (lq + 1) * PART, :], fs[:])

    nc.compile()
    return nc


_NC_CACHE = None


def _get_nc():
    global _NC_CACHE
    if _NC_CACHE is None:
        _NC_CACHE = build_nc()
    return _NC_CACHE


def make_in_maps(x, attn_mask, w_qkv, b_qkv, w_out, b_out):
    """Host-side sharding + layout prep -> per-core input maps."""
    bf16 = ml_dtypes.bfloat16
    x = np.asarray(x, dtype=np.float32)
    attn_mask = np.asarray(attn_mask)
    w_qkv = np.asarray(w_qkv, dtype=np.float32)
    b_qkv = np.asarray(b_qkv, dtype=np.float32)
    w_out = np.asarray(w_out, dtype=np.float32)
    b_out = np.asarray(b_out, dtype=np.float32)

    # wqk[m, p, k*128+c] = w_qkv[k*128+p, m*128+c] for m-tiles 0..15 (q, k)
    wqk = np.ascontiguousarray(
        w_qkv[:, : 2 * D]
        .reshape(NK, PART, 2 * NK, PART)
        .transpose(2, 1, 0, 3)
        .reshape(2 * NK, PART, L)
    ).astype(bf16)
    wv = np.ascontiguousarray(w_qkv[:, 2 * D :].reshape(NK, PART, D)).astype(bf16)
    wo = np.ascontiguousarray(w_out.reshape(NK, PART, D)).astype(bf16)
    bqk_h = np.ascontiguousarray(b_qkv[: 2 * D].reshape(2 * NK, PART).T).astype(
        np.float32
    )
    bv_h = np.ascontiguousarray(np.broadcast_to(b_qkv[2 * D :], (PART, D))).astype(
        np.float32
    )
    boutb = np.ascontiguousarray(np.broadcast_to(b_out, (PART, D))).astype(np.float32)
    maskbias = np.where(attn_mask.astype(bool), 0.0, -10000.0).astype(np.float32)

    sel_host = np.zeros((2, PART), dtype=np.float32)
    sel_host[0, 0:HD] = 1.0
    sel_host[1, HD:PART] = 1.0
    sel_host = sel_host.astype(bf16)

    in_maps = []
    for b in range(B):
        xTb = np.ascontiguousarray(x[b].T.reshape(NK, PART, L)).astype(bf16)
        mb = np.ascontiguousarray(maskbias[b].reshape(NLK, PART).T).astype(np.float32)
        in_maps.append(
            {
                "xT": xTb,
                "wqk": wqk,
                "wv": wv,
                "wo": wo,
                "bqk": bqk_h,
                "bv": bv_h,
                "bout": boutb,
                "maskb": mb,
                "sel": sel_host,
            }
        )
    return in_maps


def kernel(x, attn_mask, w_qkv, b_qkv, w_out, b_out):
    in_maps = make_in_maps(x, attn_mask, w_qkv, b_qkv, w_out, b_out)
    nc = _get_nc()
    res = run_bass_kernel_spmd(nc, in_maps, core_ids=list(range(N_CORES)))
    return np.stack([res.results[b]["Y"] for b in range(B)], axis=0)


if __name__ == "__main__":
    rng = np.random.default_rng(0)
    inputs = {
        "x": rng.standard_normal((B, L, D), dtype=np.float32),
        "attn_mask": np.ones((B, L), dtype=bool),
        "w_qkv": ((rng.random((D, D3), dtype=np.float32) - 0.5) / 16.0),
        "b_qkv": np.zeros((D3,), dtype=np.float32),
        "w_out": ((rng.random((D, D), dtype=np.float32) - 0.5) / 16.0),
        "b_out": np.zeros((D,), dtype=np.float32),
    }
    y = kernel(**inputs)
    print(y.shape, y.dtype)
